# revision 2
# baseline (speedup 1.0000x reference)
"""Causal self-attention on 8 Trainium2 NeuronCores — v2 (interleaved).

Sharding: core c = (batch b = c//2) x (head-half h2 = c%2), as baseline.

v2 changes vs baseline:
  * all matmul operands bf16 (moving-operand width 1024, FWL weight loads,
    half the DMA bytes); PSUM accumulation stays f32, exp input f32.
  * single emission stream that software-pipelines head-pair jp's QKV
    projection INTO the attention of pair jp-1, so TensorE fills the gaps
    the ACT-paced softmax leaves (ACT exp ~172us < PE ~250us per core).
    The QKV chains share the attention S-tile PSUM slots (tag rotation
    handles WAR deps); PSUM stays within 8 banks.
  * out-projection reads bf16 ot tiles; y returned bf16, host does the
    cross-core partial add + bias in f32.

Per-core engine budget (cycles @2.4GHz PE / 1.2GHz ACT):
  PE: QKV 197k + S 156k + PV 164k + proj 66k = 583k  (~243us)
  ACT: exp ~139k cols + 192*352 overhead              (~172us)
  => PE-bound if occupancy is high; target ~260us sim.
"""
import sys

sys.path.insert(0, "/opt/trn_rl_repo")

import numpy as np

import concourse.bacc as bacc
import concourse.mybir as mybir
import concourse.tile as tile
from concourse.bass_utils import run_bass_kernel_spmd

B, T, C = 4, 2048, 1024
H = 16
HD = C // H              # 64
N_CORES = 8
HL = H // 2              # 8 local heads per core
CL = HL * HD             # 512 local channels
F32 = mybir.dt.float32
BF16 = mybir.dt.bfloat16

QG = 1024                # q-group width
NQG = T // QG            # 2
KB = 128                 # k-block
NCCH = C // 128          # 8 contraction chunks

_cache = {}


def _build(dbg=False, reps=1):
    nc = bacc.Bacc("TRN2", target_bir_lowering=False, debug=False,
                   num_devices=N_CORES)

    xT = nc.dram_tensor("xT", [C, T], BF16, kind="ExternalInput")
    wqk = nc.dram_tensor("wqk", [C, 2 * CL], BF16, kind="ExternalInput")
    wv = nc.dram_tensor("wv", [C, CL], BF16, kind="ExternalInput")
    wout = nc.dram_tensor("wout", [CL, C], BF16, kind="ExternalInput")
    mask = nc.dram_tensor("mask", [KB, KB], BF16, kind="ExternalInput")
    ident = nc.dram_tensor("ident", [KB, KB], BF16, kind="ExternalInput")
    y = nc.dram_tensor("y", [T, C], BF16, kind="ExternalOutput")

    with tile.TileContext(nc) as tc:
      for _rep in range(reps):
        with tc.tile_pool(name="persist", bufs=1) as pp:
            # persistent SBUF tiles
            xt_t = [pp.tile([128, T], BF16, tag=f"xt{i}", name=f"xt{i}")
                    for i in range(NCCH)]
            wqk_t = [pp.tile([128, 2 * CL], BF16, tag=f"wqk{i}", name=f"wqk{i}")
                     for i in range(NCCH)]
            wv_t = [pp.tile([128, CL], BF16, tag=f"wv{i}", name=f"wv{i}")
                    for i in range(NCCH)]
            wout_t = [pp.tile([128, C], BF16, tag=f"wo{j}", name=f"wo{j}")
                      for j in range(4)]
            ot = [pp.tile([128, T], BF16, tag=f"ot{j}", name=f"ot{j}")
                  for j in range(4)]
            msk = pp.tile([KB, KB], BF16, tag="msk", name="msk")
            idn = pp.tile([KB, KB], BF16, tag="idn", name="idn")

            # input DMAs: (xt, wqk) chunk pairs first so the first QKV
            # chains unblock progressively, then wv / wout / mask.
            for i in range(NCCH):
                nc.sync.dma_start(xt_t[i][:], xT[i * 128:(i + 1) * 128, :])
                nc.sync.dma_start(wqk_t[i][:], wqk[i * 128:(i + 1) * 128, :])
            for i in range(NCCH):
                nc.sync.dma_start(wv_t[i][:], wv[i * 128:(i + 1) * 128, :])
            for j in range(4):
                nc.sync.dma_start(wout_t[j][:], wout[j * 128:(j + 1) * 128, :])
            nc.sync.dma_start(msk[:], mask[:])
            nc.sync.dma_start(idn[:], ident[:])

            with (
                tc.tile_pool(name="ps", bufs=2, space="PSUM") as ps,
                tc.tile_pool(name="qk", bufs=2) as qkp,
                tc.tile_pool(name="vw", bufs=2) as vwp,
                tc.tile_pool(name="pb", bufs=4) as pbp,
                tc.tile_pool(name="nm", bufs=2) as nmp,
            ):
                # ---------- step generators ----------
                vws = [None] * (T // KB)   # [128, 8*65] all-pair V' tiles

                def v_steps():
                    """V' for ALL head-pairs: 16 chains of [128t, 512v]."""
                    steps = []

                    def mk_v(m):
                        def fn():
                            acc = ps.tile([128, QG], F32, tag="s", name="vacc")
                            for i in range(NCCH):
                                nc.tensor.matmul(
                                    acc[:, 0:CL],
                                    xt_t[i][:, m * KB:(m + 1) * KB],
                                    wv_t[i][:],
                                    start=(i == 0), stop=(i == NCCH - 1))
                            vt = pp.tile([128, HL * (HD + 1)], BF16,
                                         tag=f"vw{m}", name=f"vw{m}")
                            vws[m] = vt
                            dst = vt[:].rearrange("p (h x) -> p h x", x=HD + 1)
                            nc.vector.tensor_copy(
                                dst[:, :, 0:HD],
                                acc[:, 0:CL].rearrange(
                                    "p (h d) -> p h d", d=HD))
                            nc.vector.memset(dst[:, :, HD:HD + 1], 1.0)
                        return fn
                    for m in range(T // KB):
                        steps.append((8 * CL, mk_v(m)))
                    return steps

                def qk_steps(jp):
                    """Q/K projection for head-pair jp. Returns
                    (steps, handles) where handles = [qp, kp] lazily set."""
                    steps = []
                    hq = [None, None]

                    def mk_qk(which, tg):
                        def fn():
                            if hq[which] is None:
                                nm = "qp" if which == 0 else "kp"
                                hq[which] = qkp.tile([128, T], BF16,
                                                     tag=nm, name=nm)
                            dst = hq[which]
                            cb = jp * 128 + which * CL
                            acc = ps.tile([128, QG], F32, tag="s", name="qkacc")
                            for half in range(2):
                                h0 = half * 512
                                for i in range(NCCH):
                                    nc.tensor.matmul(
                                        acc[:, h0:h0 + 512],
                                        wqk_t[i][:, cb:cb + 128],
                                        xt_t[i][:, tg * QG + h0:
                                                tg * QG + h0 + 512],
                                        start=(i == 0), stop=(i == NCCH - 1))
                            nc.vector.tensor_copy(
                                dst[:, tg * QG:(tg + 1) * QG], acc[:])
                        return fn
                    for which in range(2):
                        for tg in range(NQG):
                            steps.append((8 * QG, mk_qk(which, tg)))
                    return steps, hq

                def att_steps(jp, hq, g_list=None):
                    """Attention kb-steps for head-pair jp. The two heads
                    of the pair advance in lockstep per kb, so their exps
                    alternate on ACT and each stream's PSUM-slot recycle
                    latency hides behind the other stream's exp."""
                    steps = []

                    def emit_s(kb, qlo, pb):
                        r0 = max(0, kb * KB - qlo)
                        diag = kb * KB >= qlo
                        s_ps = ps.tile([128, QG], F32, tag="s", name="sps")
                        c0 = r0
                        while c0 < QG:
                            c1 = min(QG, (c0 // 512 + 1) * 512)
                            nc.tensor.matmul(
                                s_ps[:, c0:c1],
                                hq[1][pb:pb + 64, kb * KB:(kb + 1) * KB],
                                hq[0][pb:pb + 64, qlo + c0:qlo + c1],
                                start=True,
                                stop=(not diag) or (c0 != r0))
                            c0 = c1
                        if diag:
                            nc.tensor.matmul(
                                s_ps[:, r0:r0 + KB], idn[:], msk[:],
                                start=False, stop=True)
                        p_sb = pbp.tile([128, QG], BF16, tag="p", name="p")
                        nc.scalar.activation(
                            p_sb[:, r0:], s_ps[:, r0:],
                            mybir.ActivationFunctionType.Exp, scale=0.125)
                        return p_sb

                    def emit_pv(st, kb, p_sb, qlo, col0, nkb):
                        r0 = max(0, kb * KB - qlo)
                        lhv = vws[kb][:, col0:col0 + HD + 1]
                        c0 = (r0 // 512) * 512
                        while c0 < QG:
                            c1 = min(QG, c0 + 512)
                            rs = max(c0, r0)
                            last_kb = min(nkb, (qlo + c1) // KB) - 1
                            nc.tensor.matmul(
                                st["o"][0:HD + 1, rs:c1],
                                lhv, p_sb[:, rs:c1],
                                start=(kb == 0), stop=(kb == last_kb))
                            c0 = c1

                    def emit_norm(st, qlo, pb):
                        o_ps = st["o"]
                        rr = nmp.tile([65, QG], F32, tag="rr", name="rr")
                        nc.vector.reciprocal(rr[64:65, :],
                                             o_ps[HD:HD + 1, :])
                        rr0 = nmp.tile([1, QG], F32, tag="rr0", name="rr0")
                        nc.sync.dma_start(rr0[:], rr[64:65, :])
                        rb = nmp.tile([64, QG], F32, tag="rb", name="rb")
                        nc.gpsimd.partition_broadcast(rb[:], rr0[:])
                        if pb == 0:
                            nc.vector.tensor_mul(
                                ot[jp][0:64, qlo:qlo + QG],
                                o_ps[0:HD, :], rb[:])
                        else:
                            os_ = nmp.tile([64, QG], BF16, tag="os",
                                           name="os")
                            nc.vector.tensor_mul(os_[:], o_ps[0:HD, :],
                                                 rb[:])
                            nc.sync.dma_start(
                                ot[jp][64:128, qlo:qlo + QG], os_[:])

                    for g in (g_list if g_list is not None else range(NQG)):
                        qlo = g * QG
                        nkb = (qlo + QG) // KB
                        sts = [{"pb": 0, "col0": (2 * jp) * (HD + 1)},
                               {"pb": 64, "col0": (2 * jp + 1) * (HD + 1)}]

                        def mk_step(kb, sts=sts, qlo=qlo, nkb=nkb):
                            def fn():
                                for st in sts:
                                    if kb == 0:
                                        st["o"] = ps.tile([128, QG], F32,
                                                          tag="o", name="ops")
                                    st["cur"] = emit_s(kb, qlo, st["pb"])
                                for st in sts:
                                    if "prev" in st:
                                        emit_pv(st, kb - 1, st["prev"],
                                                qlo, st["col0"], nkb)
                                    st["prev"] = st["cur"]
                            return fn

                        def mk_fin(sts=sts, qlo=qlo, nkb=nkb):
                            def fn():
                                for st in sts:
                                    emit_pv(st, nkb - 1, st["prev"],
                                            qlo, st["col0"], nkb)
                                    emit_norm(st, qlo, st["pb"])
                            return fn

                        for kb in range(nkb):
                            w = 2 * ((QG - max(0, kb * KB - qlo)) + 512)
                            steps.append((w, mk_step(kb)))
                        steps.append((1400, mk_fin()))
                    return steps

                def zipper(a, b):
                    """Emit two weighted step streams, keeping their
                    fractional progress aligned (cycle-proportional)."""
                    ta = sum(w for w, _ in a) or 1
                    tb = sum(w for w, _ in b) or 1
                    ia = ib = 0
                    ca = cb = 0.0
                    while ia < len(a) or ib < len(b):
                        if ib >= len(b) or (ia < len(a)
                                            and ca / ta <= cb / tb):
                            w, fn = a[ia]
                            fn(); ca += w; ia += 1
                        else:
                            w, fn = b[ib]
                            fn(); cb += w; ib += 1

                def proj_steps(ms):
                    """Output-projection chunks (PSUM via the shared
                    s-slots; DVE copy-out; DMA store)."""
                    steps = []

                    def mk_p(m):
                        def fn():
                            y_ps = ps.tile([128, QG], F32, tag="s",
                                           name="yps")
                            for n in range(2):
                                for j in range(4):
                                    nc.tensor.matmul(
                                        y_ps[:, n * 512:(n + 1) * 512],
                                        ot[j][:, m * KB:(m + 1) * KB],
                                        wout_t[j][:, n * 512:(n + 1) * 512],
                                        start=(j == 0), stop=(j == 3))
                            ysb = nmp.tile([128, C], BF16, tag="ysb",
                                           name="ysb")
                            nc.vector.tensor_copy(ysb[:], y_ps[:])
                            nc.sync.dma_start(
                                y[m * KB:(m + 1) * KB, :], ysb[:])
                        return fn
                    for m in ms:
                        steps.append((8 * 512, mk_p(m)))
                    return steps

                # ---------- emission ----------
                # prologue: pair-0 Q/K, then all-pair V
                sq, hq_prev = qk_steps(0)
                for _, fn in sq:
                    fn()
                for _, fn in v_steps():
                    fn()
                # windows 1-3: attention(jp-1) with pair-jp Q/K as filler
                for jp in range(1, 4):
                    sq, hq_next = qk_steps(jp)
                    zipper(att_steps(jp - 1, hq_prev), sq)
                    hq_prev = hq_next
                # window 4: attention(3); q-groups g=0 first (both heads),
                # then g=1 interleaved with the first half of the output
                # projection (those m-chunks only read ot[:, 0:QG]).
                for _, fn in att_steps(3, hq_prev, g_list=[0]):
                    fn()
                zipper(att_steps(3, hq_prev, g_list=[1]),
                       proj_steps(range(0, 8)))
                for _, fn in proj_steps(range(8, T // KB)):
                    fn()

    nc.compile()
    return nc


def make_in_maps(x, W_qkv, W_out):
    import ml_dtypes
    BF = ml_dtypes.bfloat16
    x = np.asarray(x, dtype=np.float32)
    W_qkv = np.asarray(W_qkv, dtype=np.float32)
    W_out = np.asarray(W_out, dtype=np.float32)
    mask = np.where(
        np.arange(KB)[None, :] < np.arange(KB)[:, None], -1e30, 0.0
    ).astype(BF)
    ident = np.eye(KB).astype(BF)
    in_maps = []
    for c in range(N_CORES):
        b, h2 = c // 2, c % 2
        cols = slice(h2 * CL, (h2 + 1) * CL)
        in_maps.append({
            "xT": np.ascontiguousarray(x[b].T).astype(BF),
            "wqk": np.ascontiguousarray(
                np.concatenate([W_qkv[:, cols],
                                W_qkv[:, C:][:, cols]], axis=1)).astype(BF),
            "wv": np.ascontiguousarray(W_qkv[:, 2 * C:][:, cols]).astype(BF),
            "wout": np.ascontiguousarray(W_out[cols, :]).astype(BF),
            "mask": mask,
            "ident": ident,
        })
    return in_maps


def kernel(x, W_qkv, b_qkv, W_out, b_out, _trace=False):
    b_qkv = np.asarray(b_qkv, dtype=np.float32)
    b_out = np.asarray(b_out, dtype=np.float32)
    W_out = np.asarray(W_out, dtype=np.float32)

    # q/k biases would need device-side adds; this problem pins them to 0.
    assert not b_qkv[:2 * C].any(), "nonzero q/k bias unsupported"

    if "nc" not in _cache:
        _cache["nc"] = _build()
    nc = _cache["nc"]

    in_maps = make_in_maps(x, W_qkv, W_out)

    kwargs = {}
    if _trace:
        kwargs = {"trace": True, "trace_cores": [0]}
    res = run_bass_kernel_spmd(nc, in_maps, core_ids=list(range(N_CORES)),
                               **kwargs)

    out = np.empty((B, T, C), dtype=np.float32)
    # v-bias passes through softmax as +b_v, so it folds into the output
    # projection; b_out likewise. Both are host-side adds on the partials.
    bias = b_qkv[2 * C:] @ W_out + b_out
    for b in range(B):
        out[b] = (res.results[2 * b]["y"].astype(np.float32)
                  + res.results[2 * b + 1]["y"].astype(np.float32) + bias)
    if _trace:
        kernel.last_exec_ns = res.exec_time_ns
        kernel.last_trace = (res.instructions_and_trace or (None, None))[1]
    return out
